# revision 19
# baseline (speedup 1.0000x reference)
"""Distributed Trainium2 Bass kernel for nn_AttentionLayer_25993142075512.

Sharding: 8 cores = 2 batches x 4 head-groups (4 heads each). Each core
computes its batch's q/k/v projections for its 4 heads, causal attention,
and a partial output projection o @ Wo[head_rows]. Host sums the 4
partials per batch and adds bo. No on-device collectives.

v2 design notes (vs baseline):
  - Fused phase pipeline: projections stream per 512-col sub; attention
    runs g-outer (q-group) with v'-build, scores, exp, o-accum, softmax
    normalization and the output projection all interleaved per group, so
    the PE never idles and stays at max p-state clock.
  - Causal diagonal mask applied on the PE: an extra accumulate matmul
    (-2000*I) @ tri into the score PSUM (start=False) replaces per-block
    DVE adds.
  - exp is split: diagonal tiles (and 1/4 of off-diag) use Scalar ACT
    exp; the rest use a one-op DVE Schraudolph exp -> int16 whose bits
    ARE the bf16 probs (bitcast feeds the o-matmul directly).
  - RoPE: head dims permuted host-side to [evens|odds|pass]; the
    rotate-partner is built by a PE permutation matmul, then 3 full-width
    DVE ops (t=shuf*sin, x*=cos, x+=t) per tile-sub.
  - Softmax denominators: ones column in v' (row 64 of oT). Reciprocal
    via one batched DVE reciprocal_approx_fast at partitions {0,32,64,96},
    broadcast via tiny ones-row matmuls.
  - All attention-phase PSUM lives in one 4-bank oT tile + one 4-slot
    ring shared by score tiles / v'-transposes / norm-broadcast / out-proj.
"""

import sys, os, types, ctypes, contextlib

sys.path.insert(0, "/opt/trn_rl_repo")

import numpy as np
import ml_dtypes


def _install_axon_hooks():
    so = "/opt/axon/libaxon_pjrt.so"

    def _hook_factory(so_path):
        if not os.path.exists(so_path):
            return None
        lib = ctypes.CDLL(so_path)
        if not hasattr(lib, "axon_start_nrt_profile"):
            return None
        lib.axon_start_nrt_profile.argtypes = [
            ctypes.POINTER(ctypes.c_int64),
            ctypes.c_size_t,
        ]
        lib.axon_start_nrt_profile.restype = ctypes.c_int64
        lib.axon_stop_nrt_profile.argtypes = [ctypes.c_char_p]
        lib.axon_stop_nrt_profile.restype = ctypes.c_int64

        @contextlib.contextmanager
        def _hook(output_dir, device_ids):
            import jax

            jax.devices()
            if device_ids:
                ids = (ctypes.c_int64 * len(device_ids))(*device_ids)
                rc = lib.axon_start_nrt_profile(ids, len(device_ids))
            else:
                rc = lib.axon_start_nrt_profile(None, 0)
            if rc != 0:
                raise RuntimeError(f"axon_start_nrt_profile rc={rc}")
            try:
                yield
            finally:
                n = lib.axon_stop_nrt_profile(str(output_dir).encode())
                if n < 0:
                    raise RuntimeError(f"axon_stop_nrt_profile rc={n}")

        return _hook

    try:
        import antenv

        if "antenv.axon_hooks" not in sys.modules:
            hook = _hook_factory(so)
            mod = types.ModuleType("antenv.axon_hooks")
            mod.get_axon_ntff_profile_hook = lambda: hook
            mod.set_axon_ntff_profile_hook = lambda h: None
            antenv.axon_hooks = mod
            sys.modules["antenv.axon_hooks"] = mod
    except ImportError:
        pass
    from concourse import bass_utils

    bass_utils.upload_artifacts = lambda tmpdir: tmpdir


_install_axon_hooks()

from concourse import bass, bacc, tile, mybir  # noqa: E402

BF16 = mybir.dt.bfloat16
F32 = mybir.dt.float32
I16 = mybir.dt.int16
NPBF16 = ml_dtypes.bfloat16

B, N, DQ, DKV, H, DH, DOUT = 2, 2048, 1024, 1024, 16, 64, 1024
ROT = DH // 2  # 32
HPC = 4  # heads per core
NB = N // 128  # 16 q/k blocks
NG = NB // 4  # 4 q-block groups (packs of 4)
NSUB = 4
SW = N // NSUB  # 512
VS = 72  # v' tile stride (64 v cols + ones col + pad)

TRI_C = -2000.0  # causal-mask additive constant (safe for int16 exp path)
SCH_A = 128.0 / np.log(2.0)  # Schraudolph bf16 scale
SCH_C = 8.5  # Schraudolph magic (calibrated)
MASK_BIAS = -30000.0  # scalar-path masked bias


def build_nc():
    nc = bacc.Bacc(None, target_bir_lowering=False)

    sqt = nc.declare_dram_parameter("sqt", [NSUB, 8, 128, SW], BF16, isOutput=False)
    skvt = nc.declare_dram_parameter("skvt", [NSUB, 8, 128, SW], BF16, isOutput=False)
    wq = nc.declare_dram_parameter("wq", [8, 128, HPC * DH], BF16, isOutput=False)
    wkv = nc.declare_dram_parameter("wkv", [8, 128, HPC * 2 * DH], BF16, isOutput=False)
    wo = nc.declare_dram_parameter("wo", [2, 128, DOUT], BF16, isOutput=False)
    bq = nc.declare_dram_parameter("bq", [2, 128, 1], F32, isOutput=False)
    bkv = nc.declare_dram_parameter("bkv", [HPC, 128, 1], F32, isOutput=False)
    cost_d = nc.declare_dram_parameter("cost", [128, N], BF16, isOutput=False)
    sint_d = nc.declare_dram_parameter("sint", [128, N], BF16, isOutput=False)
    perm_d = nc.declare_dram_parameter("permm", [128, 128], BF16, isOutput=False)
    negi_d = nc.declare_dram_parameter("negi", [128, 128], BF16, isOutput=False)
    tri_d = nc.declare_dram_parameter("tri", [128, 128], BF16, isOutput=False)
    bmask_d = nc.declare_dram_parameter("bmask", [NB, 128, 1], F32, isOutput=False)
    b2_d = nc.declare_dram_parameter("b2", [NB, 128, 1], F32, isOutput=False)
    out_ext = nc.declare_dram_parameter("out", [N, DOUT], BF16, isOutput=True)

    AF = mybir.ActivationFunctionType
    ALU = mybir.AluOpType

    with tile.TileContext(nc) as tc:
        with (
            tc.tile_pool(name="const", bufs=1) as cpool,
            tc.tile_pool(name="big", bufs=1) as bigpool,
            tc.tile_pool(name="stream", bufs=1) as spool,
            tc.tile_pool(name="ptile", bufs=1) as ppool,
            tc.tile_pool(name="small", bufs=1) as smallpool,
            tc.tile_pool(name="psA", bufs=1, space=bass.MemorySpace.PSUM) as psA,
            tc.tile_pool(name="psB", bufs=1, space=bass.MemorySpace.PSUM) as psB,
        ):
            # ---- constant tiles ----
            wq_sb = [cpool.tile([128, HPC * DH], BF16, tag=f"wq{c}", name=f"wq{c}") for c in range(8)]
            wkv_sb = [cpool.tile([128, HPC * 2 * DH], BF16, tag=f"wkv{c}", name=f"wkv{c}") for c in range(8)]
            wo_sb = [cpool.tile([128, DOUT], BF16, tag=f"wo{pr}", name=f"wo{pr}") for pr in range(2)]
            bq_sb = cpool.tile([128, 2], F32, tag="bq", name="bq")
            bkv_sb = cpool.tile([128, HPC], F32, tag="bkv", name="bkv")
            cost = cpool.tile([128, N], BF16, tag="cost", name="cost")
            sint = cpool.tile([128, N], BF16, tag="sint", name="sint")
            permm = cpool.tile([128, 128], BF16, tag="permm", name="permm")
            negi = cpool.tile([128, 128], BF16, tag="negi", name="negi")
            tri = cpool.tile([128, 128], BF16, tag="tri", name="tri")
            bmask = cpool.tile([128, NB], F32, tag="bmask", name="bmask")
            b2 = cpool.tile([128, NB], F32, tag="b2", name="b2")
            ones1 = cpool.tile([33, 64], BF16, tag="ones1", name="ones1")
            dn = cpool.tile([33, 2, 512], F32, tag="dn", name="dn")

            # ---- persistent activations ----
            qT = [bigpool.tile([128, N], BF16, tag=f"qT{i}", name=f"qT{i}") for i in range(2)]
            kvT = [bigpool.tile([128, N], BF16, tag=f"kvT{h}", name=f"kvT{h}") for h in range(HPC)]
            vg = [
                [bigpool.tile([128, VS], BF16, tag=f"vg{h}_{kb}", name=f"vg{h}_{kb}") for kb in range(NB)]
                for h in range(HPC)
            ]
            oTs = [
                [bigpool.tile([128, 512], BF16, tag=f"oTs{pr}_{g}", name=f"oTs{pr}_{g}") for g in range(NG)]
                for pr in range(2)
            ]

            # PSUM plan (8 banks): oT 4 banks; sT ring 2; shared ring 2
            # (proj chains, rope shuffles, v-transposes, norm broadcast, out-proj).
            oT = psA.tile([65, HPC, 512], F32, tag="oT", name="oT")

            def st_ring():
                return psA.tile([128, 512], F32, tag="sT", name="sT", bufs=3)

            def pj_ring():
                return psB.tile([128, 512], F32, tag="pj", name="pj", bufs=1)

            # ---- DMAs: critical-path first ----
            nc.sync.dma_start(bq_sb[:, 0:1], bq[0])
            nc.sync.dma_start(bq_sb[:, 1:2], bq[1])
            for h in range(HPC):
                nc.sync.dma_start(bkv_sb[:, h : h + 1], bkv[h])

            xqs = [[None] * 8 for _ in range(NSUB)]
            xkvs = [[None] * 8 for _ in range(NSUB)]

            def dma_x(sub, kv):
                for c in range(8):
                    t = spool.tile(
                        [128, SW], BF16, tag="xkv" if kv else "xq", name="x", bufs=32
                    )
                    src = skvt if kv else sqt
                    nc.sync.dma_start(t[:], src[sub, c])
                    if kv:
                        xkvs[sub][c] = t
                    else:
                        xqs[sub][c] = t

            for c in range(8):
                nc.sync.dma_start(wq_sb[c][:], wq[c])
                t = spool.tile([128, SW], BF16, tag="xq", name="x", bufs=32)
                nc.sync.dma_start(t[:], sqt[0, c])
                xqs[0][c] = t
            nc.sync.dma_start(permm[:], perm_d[:])
            nc.sync.dma_start(cost[:], cost_d[:])
            nc.sync.dma_start(sint[:], sint_d[:])
            for c in range(8):
                nc.sync.dma_start(wkv_sb[c][:], wkv[c])
                t = spool.tile([128, SW], BF16, tag="xkv", name="x", bufs=32)
                nc.sync.dma_start(t[:], skvt[0, c])
                xkvs[0][c] = t
            nc.sync.dma_start(negi[:], negi_d[:])
            nc.sync.dma_start(tri[:], tri_d[:])
            for kb in range(NB):
                nc.sync.dma_start(bmask[:, kb : kb + 1], bmask_d[kb])
                nc.sync.dma_start(b2[:, kb : kb + 1], b2_d[kb])
            dma_x(1, False)
            dma_x(1, True)
            for pr in range(2):
                nc.sync.dma_start(wo_sb[pr][:], wo[pr])
            nc.vector.memset(ones1[:], 1.0)
            nc.vector.memset(dn[:], 1.0)
            for h in range(HPC):
                for kb in range(NB):
                    nc.gpsimd.memset(vg[h][kb][:], 1.0)
            for sub in range(2, NSUB):
                dma_x(sub, False)
                dma_x(sub, True)

            # ---------------- emission building blocks ----------------
            def rope_unit(dst, rbase, rows, cs):
                """RoPE in place on dst[rbase:rbase+rows, cs]."""
                sh = pj_ring()
                rsl = slice(rbase, rbase + rows)
                nc.tensor.matmul(
                    sh[rsl, 0:SW],
                    permm[rsl, rbase : rbase + rows],
                    dst[rsl, cs],
                    start=True,
                    stop=True,
                )
                tsb = smallpool.tile([128, SW], BF16, tag="ropet", name="ropet", bufs=2)
                tsc = smallpool.tile([128, SW], BF16, tag="ropec", name="ropec", bufs=2)
                nc.vector.tensor_mul(tsb[rsl, :], sh[rsl, 0:SW], sint[rsl, cs])
                nc.gpsimd.tensor_mul(tsc[rsl, :], dst[rsl, cs], cost[rsl, cs])
                nc.vector.tensor_add(dst[rsl, cs], tsc[rsl, :], tsb[rsl, :])

            def proj_chain(sub, kind, i):
                """Emit one projection chain (8 matmuls + act + rope)."""
                nq0 = sub * SW
                cs = slice(nq0, nq0 + SW)
                ps = pj_ring()
                for c in range(8):
                    w_sb = wq_sb[c] if kind == "q" else wkv_sb[c]
                    x_sb = xqs[sub][c] if kind == "q" else xkvs[sub][c]
                    nc.tensor.matmul(
                        ps[:],
                        w_sb[:, i * 128 : (i + 1) * 128],
                        x_sb[:],
                        start=(c == 0),
                        stop=(c == 7),
                    )
                if kind == "q":
                    nc.scalar.activation(
                        qT[i][:, cs], ps[:], AF.Identity, bias=bq_sb[:, i : i + 1]
                    )
                    rope_unit(qT[i], 0, 128, cs)
                else:
                    nc.scalar.activation(
                        kvT[i][:, cs], ps[:], AF.Identity, bias=bkv_sb[:, i : i + 1]
                    )
                    rope_unit(kvT[i], (i % 2) * 64, 64, cs)
                    vb = 64 if i % 2 == 0 else 0
                    for j in range(4):
                        kb = 4 * sub + j
                        nc.scalar.dma_start(
                            vg[i][kb][:, 0:64],
                            kvT[i][vb : vb + 64, kb * 128 : (kb + 1) * 128],
                            transpose=True,
                        )

            exp_cnt = [0]

            pend_o = []  # staggered o-matmul emitters (one tile behind)

            def flush_o(n=None):
                k = len(pend_o) if n is None else min(n, len(pend_o))
                for _ in range(k):
                    pend_o.pop(0)()

            def att_tile(g, h, kb):
                """Score + exp now; o-accumulate staggered one tile later."""
                kr = (h % 2) * 64
                pr, hr = h // 2, (h % 2) * 64
                diag = kb // 4 == g
                off = (kb % 4) * 128 if diag else 0
                w = 512 - off
                sT = st_ring()
                nc.tensor.matmul(
                    sT[:, off : off + w],
                    kvT[h][kr : kr + 64, kb * 128 : (kb + 1) * 128],
                    qT[pr][hr : hr + 64, g * 512 + off : g * 512 + off + w],
                    start=True,
                    stop=not diag,
                )
                if diag:
                    nc.tensor.matmul(
                        sT[:, off : off + 128], negi[:], tri[:], start=False, stop=True
                    )
                use_scalar = diag or (exp_cnt[0] % 2 == 0)
                if not diag:
                    exp_cnt[0] += 1
                if use_scalar:
                    p = ppool.tile([128, 512], BF16, tag="p", name="p", bufs=4)
                    nc.scalar.activation(
                        p[:, off : off + w],
                        sT[:, off : off + w],
                        AF.Exp,
                        bias=bmask[:, kb : kb + 1],
                        scale=0.125,
                    )
                    pmv = p[:, off : off + w]
                else:
                    pi = ppool.tile([128, 512], I16, tag="pi", name="pi", bufs=4)
                    nc.vector.tensor_scalar(
                        pi[:, off : off + w],
                        sT[:, off : off + w],
                        0.125 * SCH_A,
                        b2[:, kb : kb + 1],
                        ALU.mult,
                        ALU.add,
                    )
                    pmv = pi[:, off : off + w].bitcast(BF16)

                nc.tensor.matmul(
                    oT[:, h, off : off + w],
                    vg[h][kb][:, 0:65],
                    pmv,
                    start=(kb == 0),
                    stop=(kb == 4 * g + 3),
                )
                if kb == 4 * g + 3:  # head closed: stage denominator row
                    r0 = 32 * (h // 2)
                    nc.vector.tensor_copy(dn[r0 : r0 + 1, h % 2, :], oT[64:65, h, :])

            def norm_group(g):
                flush_o()
                rec = smallpool.tile([33, 2, 512], F32, tag="rec", name="rec", bufs=2)
                recb = smallpool.tile([33, 2, 512], BF16, tag="recb", name="recb", bufs=2)
                nc.vector.reciprocal_approx_fast(rec[:], dn[:])
                nc.vector.tensor_copy(recb[:], rec[:])
                for h in range(HPC):
                    pr, hr = h // 2, (h % 2) * 64
                    r0 = 32 * (h // 2)
                    bc = pj_ring()
                    nc.tensor.matmul(
                        bc[0:64, :],
                        ones1[r0 : r0 + 1, :],
                        recb[r0 : r0 + 1, h % 2, :],
                        start=True,
                        stop=True,
                    )
                    bcs = smallpool.tile([64, 512], F32, tag="bcs", name="bcs", bufs=2)
                    nc.scalar.activation(bcs[:], bc[0:64, :], AF.Copy)
                    nc.vector.tensor_mul(
                        oTs[pr][g][hr : hr + 64, :], oT[0:64, h, :], bcs[:]
                    )

            def po_unit(g, qb, nh, use_st=False):
                def emit():
                    off = (qb % 4) * 128
                    po = st_ring() if use_st else pj_ring()
                    for pr in range(2):
                        nc.tensor.matmul(
                            po[:],
                            oTs[pr][g][:, off : off + 128],
                            wo_sb[pr][:, nh * 512 : (nh + 1) * 512],
                            start=(pr == 0),
                            stop=(pr == 1),
                        )
                    ob = smallpool.tile([128, 512], BF16, tag="ob", name="ob", bufs=4)
                    if (qb + nh) % 2 == 0:
                        nc.scalar.activation(ob[:], po[:], AF.Copy)
                    else:
                        nc.vector.tensor_copy(ob[:], po[:])
                    nc.sync.dma_start(
                        out_ext[qb * 128 : (qb + 1) * 128, nh * 512 : (nh + 1) * 512],
                        ob[:],
                    )
                return emit

            def att_units(g, heads):
                """Emitter list for attention of group g over given heads,
                each head prefixed by its v'-build."""
                units = []
                for h in heads:
                    for kb in range(4 * g + 4):
                        units.append(lambda g=g, h=h, kb=kb: att_tile(g, h, kb))
                return units

            def emit_all(units):
                for u in units:
                    u()

            def interleave(chains, fillers):
                """Emit chain emitters with filler units spread between them."""
                fillers = list(fillers)
                nch = len(chains)
                per = max(1, len(fillers) // max(1, nch)) if fillers else 0
                fi = 0
                for ci, ch in enumerate(chains):
                    ch()
                    take = per if ci < nch - 1 else len(fillers) - fi
                    for _ in range(max(0, take)):
                        if fi < len(fillers):
                            fillers[fi]()
                            fi += 1
                while fi < len(fillers):
                    fillers[fi]()
                    fi += 1

            # ---------------- fused pipeline ----------------
            def chains_for(sub):
                order = [("q", 0), ("kv", 0), ("kv", 1), ("q", 1), ("kv", 2), ("kv", 3)]
                return [lambda k=k, i=i: proj_chain(sub, k, i) for k, i in order]

            # sub 0: chains serialized against DMA arrival; attention g0
            # tiles become available per head as its chains complete.
            a0 = att_units(0, range(HPC))
            interleave(chains_for(0), [])
            emit_all(a0)
            norm_group(0)
            # proj sub1 with po(g0) as fillers (via sT slots; no att tiles live)
            interleave(
                chains_for(1),
                [po_unit(0, qb, nh, use_st=True) for qb in range(0, 4) for nh in range(2)],
            )
            a1 = att_units(1, range(HPC))
            half = len(a1) // 2
            emit_all(a1[:half])
            interleave(chains_for(2), a1[half:])
            norm_group(1)
            a2 = att_units(2, range(HPC))
            third = len(a2) // 3
            interleave([po_unit(1, qb, nh) for qb in range(4, 8) for nh in range(2)], a2[:third])
            interleave(chains_for(3), a2[third:])
            norm_group(2)
            a3 = att_units(3, range(HPC))
            interleave([po_unit(2, qb, nh) for qb in range(8, 12) for nh in range(2)], a3)
            norm_group(3)
            for qb in range(12, 16):
                for nh in range(2):
                    po_unit(3, qb, nh, use_st=(nh % 2 == 1))()

    nc.compile()
    return nc


def _head_perm():
    """Permute one head's 64 dims: [evens(0,2..30), odds(1,3..31), pass 32:64]."""
    ev = np.arange(0, ROT, 2)
    od = np.arange(1, ROT, 2)
    return np.concatenate([ev, od, np.arange(ROT, DH)])


def _prep_host(s_q, s_kv, mask_q, mask_kv, Wq, bq_, Wkv, bkv_, Wo, bo_):
    """Build per-core input maps (host-side shard + transform)."""
    perm = _head_perm()

    # RoPE tables [128, N]: per 64-row block: rows 0:16 evens (cos, +sin),
    # rows 16:32 odds (cos, -sin), rows 32:64 pass (1, 0).
    inv_freq = 1.0 / (10000.0 ** (np.arange(0, ROT, 2, dtype=np.float64) / ROT))
    t = np.arange(N, dtype=np.float64)[None, :] * inv_freq[:, None]  # [16, N]
    cosT = np.zeros((128, N), np.float32)
    sinT = np.zeros((128, N), np.float32)
    cosT[:, :] = 1.0
    for rb in (0, 64):
        cosT[rb : rb + 16] = np.cos(t)
        cosT[rb + 16 : rb + 32] = np.cos(t)
        sinT[rb : rb + 16] = -np.sin(t)
        sinT[rb + 16 : rb + 32] = np.sin(t)
    cosT = cosT.astype(NPBF16)
    sinT = sinT.astype(NPBF16)

    # partner permutation matrix: permm[r, p] = 1 iff r = partner(p)
    pm = np.zeros((128, 128), np.float32)
    for p in range(128):
        b = p % 64
        if b < 16:
            partner = p + 16
        elif b < 32:
            partner = p - 16
        else:
            partner = p
        pm[partner, p] = 1.0
    pm = pm.astype(NPBF16)

    negi = (TRI_C * np.eye(128, dtype=np.float32)).astype(NPBF16)
    pidx = np.arange(128)
    trim = (pidx[:, None] > pidx[None, :]).astype(np.float32).astype(NPBF16)

    in_maps = []
    for core in range(8):
        b = core // 4
        h0 = (core % 4) * HPC

        wq_cols = []
        bq_cols = []
        for h in range(h0, h0 + HPC):
            wq_cols.append(Wq[:, h * DH : (h + 1) * DH][:, perm])
            bq_cols.append(bq_[h * DH : (h + 1) * DH][perm])
        wq_c = np.concatenate(wq_cols, axis=1)  # [1024, 256]
        bq_c = np.concatenate(bq_cols)  # [256]

        wkv_cols = []
        bkv_cols = []
        for h in range(h0, h0 + HPC):
            kcols = Wkv[:, h * 2 * DH : h * 2 * DH + DH][:, perm]
            vcols = Wkv[:, h * 2 * DH + DH : (h + 1) * 2 * DH]
            kb_ = bkv_[h * 2 * DH : h * 2 * DH + DH][perm]
            vb_ = bkv_[h * 2 * DH + DH : (h + 1) * 2 * DH]
            if (h - h0) % 2 == 0:  # even head: [k; v]
                wkv_cols.append(np.concatenate([kcols, vcols], axis=1))
                bkv_cols.append(np.concatenate([kb_, vb_]))
            else:  # odd head: [v; k] so k-rows sit at partition base 64
                wkv_cols.append(np.concatenate([vcols, kcols], axis=1))
                bkv_cols.append(np.concatenate([vb_, kb_]))
        wkv_c = np.concatenate(wkv_cols, axis=1)  # [1024, 512]

        wo_rows = Wo[h0 * DH : (h0 + HPC) * DH, :]  # [256, 1024]

        braw = 1.0e6 * (mask_kv[b].astype(np.float64) - 1.0)  # 0 or -1e6
        bmask = np.clip(braw, MASK_BIAS, 0.0).reshape(NB, 128, 1)
        b2v = (
            np.clip(SCH_A * braw, -40000.0, 0.0) + (127.0 * 128.0 - SCH_C)
        ).reshape(NB, 128, 1)

        in_maps.append(
            {
                "sqt": np.ascontiguousarray(
                    s_q[b].T.reshape(1024, NSUB, SW).transpose(1, 0, 2).reshape(NSUB, 8, 128, SW)
                ).astype(NPBF16),
                "skvt": np.ascontiguousarray(
                    s_kv[b].T.reshape(1024, NSUB, SW).transpose(1, 0, 2).reshape(NSUB, 8, 128, SW)
                ).astype(NPBF16),
                "wq": np.ascontiguousarray(wq_c.reshape(8, 128, HPC * DH)).astype(NPBF16),
                "wkv": np.ascontiguousarray(wkv_c.reshape(8, 128, HPC * 2 * DH)).astype(NPBF16),
                "wo": np.ascontiguousarray(wo_rows.reshape(2, 128, DOUT)).astype(NPBF16),
                "bq": bq_c.reshape(2, 128, 1).astype(np.float32),
                "bkv": np.stack(bkv_cols).reshape(HPC, 128, 1).astype(np.float32),
                "cost": cosT,
                "sint": sinT,
                "permm": pm,
                "negi": negi,
                "tri": trim,
                "bmask": bmask.astype(np.float32),
                "b2": b2v.astype(np.float32),
            }
        )
    return in_maps


_NC_CACHE = {}


def kernel(s_q, s_kv, mask_q, mask_kv, Wq, bq, Wkv, bkv, Wo, bo, _return_results=False):
    from concourse.bass_utils import run_bass_kernel_spmd

    if "nc" not in _NC_CACHE:
        _NC_CACHE["nc"] = build_nc()
    nc = _NC_CACHE["nc"]

    in_maps = _prep_host(
        np.asarray(s_q, np.float32),
        np.asarray(s_kv, np.float32),
        np.asarray(mask_q, np.float32),
        np.asarray(mask_kv, np.float32),
        np.asarray(Wq, np.float32),
        np.asarray(bq, np.float32),
        np.asarray(Wkv, np.float32),
        np.asarray(bkv, np.float32),
        np.asarray(Wo, np.float32),
        np.asarray(bo, np.float32),
    )
    trace = bool(int(os.environ.get("KERNEL_TRACE", "0")))
    res = run_bass_kernel_spmd(nc, in_maps, core_ids=list(range(8)), trace=trace)

    out = np.zeros((B, N, DOUT), np.float32)
    for core in range(8):
        b = core // 4
        out[b] += res.results[core]["out"].astype(np.float32)
    out += np.asarray(bo, np.float32)[None, None, :]
    if _return_results:
        return out, res
    return out


# revision 20
# speedup vs baseline: 1.5386x; 1.5386x over previous
"""Distributed Trainium2 Bass kernel for nn_AttentionLayer_25993142075512.

Sharding: 8 cores = 2 batches x 4 head-groups (4 heads each). Each core
computes its batch's q/k/v projections for its 4 heads, causal attention,
and a partial output projection o @ Wo[head_rows]. Host sums the 4
partials per batch and adds bo. No on-device collectives.

v2 design notes (vs baseline):
  - Fused phase pipeline: projections stream per 512-col sub; attention
    runs g-outer (q-group) with v'-build, scores, exp, o-accum, softmax
    normalization and the output projection all interleaved per group, so
    the PE never idles and stays at max p-state clock.
  - Causal diagonal mask applied on the PE: an extra accumulate matmul
    (-2000*I) @ tri into the score PSUM (start=False) replaces per-block
    DVE adds.
  - exp is split: diagonal tiles (and 1/4 of off-diag) use Scalar ACT
    exp; the rest use a one-op DVE Schraudolph exp -> int16 whose bits
    ARE the bf16 probs (bitcast feeds the o-matmul directly).
  - RoPE: head dims permuted host-side to [evens|odds|pass]; the
    rotate-partner is built by a PE permutation matmul, then 3 full-width
    DVE ops (t=shuf*sin, x*=cos, x+=t) per tile-sub.
  - Softmax denominators: ones column in v' (row 64 of oT). Reciprocal
    via one batched DVE reciprocal_approx_fast at partitions {0,32,64,96},
    broadcast via tiny ones-row matmuls.
  - All attention-phase PSUM lives in one 4-bank oT tile + one 4-slot
    ring shared by score tiles / v'-transposes / norm-broadcast / out-proj.
"""

import sys, os, types, ctypes, contextlib

sys.path.insert(0, "/opt/trn_rl_repo")

import numpy as np
import ml_dtypes


def _install_axon_hooks():
    so = "/opt/axon/libaxon_pjrt.so"

    def _hook_factory(so_path):
        if not os.path.exists(so_path):
            return None
        lib = ctypes.CDLL(so_path)
        if not hasattr(lib, "axon_start_nrt_profile"):
            return None
        lib.axon_start_nrt_profile.argtypes = [
            ctypes.POINTER(ctypes.c_int64),
            ctypes.c_size_t,
        ]
        lib.axon_start_nrt_profile.restype = ctypes.c_int64
        lib.axon_stop_nrt_profile.argtypes = [ctypes.c_char_p]
        lib.axon_stop_nrt_profile.restype = ctypes.c_int64

        @contextlib.contextmanager
        def _hook(output_dir, device_ids):
            import jax

            jax.devices()
            if device_ids:
                ids = (ctypes.c_int64 * len(device_ids))(*device_ids)
                rc = lib.axon_start_nrt_profile(ids, len(device_ids))
            else:
                rc = lib.axon_start_nrt_profile(None, 0)
            if rc != 0:
                raise RuntimeError(f"axon_start_nrt_profile rc={rc}")
            try:
                yield
            finally:
                n = lib.axon_stop_nrt_profile(str(output_dir).encode())
                if n < 0:
                    raise RuntimeError(f"axon_stop_nrt_profile rc={n}")

        return _hook

    try:
        import antenv

        if "antenv.axon_hooks" not in sys.modules:
            hook = _hook_factory(so)
            mod = types.ModuleType("antenv.axon_hooks")
            mod.get_axon_ntff_profile_hook = lambda: hook
            mod.set_axon_ntff_profile_hook = lambda h: None
            antenv.axon_hooks = mod
            sys.modules["antenv.axon_hooks"] = mod
    except ImportError:
        pass
    from concourse import bass_utils

    bass_utils.upload_artifacts = lambda tmpdir: tmpdir


_install_axon_hooks()

from concourse import bass, bacc, tile, mybir  # noqa: E402

BF16 = mybir.dt.bfloat16
F32 = mybir.dt.float32
I16 = mybir.dt.int16
NPBF16 = ml_dtypes.bfloat16

B, N, DQ, DKV, H, DH, DOUT = 2, 2048, 1024, 1024, 16, 64, 1024
ROT = DH // 2  # 32
HPC = 4  # heads per core
NB = N // 128  # 16 q/k blocks
NG = NB // 4  # 4 q-block groups (packs of 4)
NSUB = 4
SW = N // NSUB  # 512
VS = 72  # v' tile stride (64 v cols + ones col + pad)

TRI_C = -2000.0  # causal-mask additive constant (safe for int16 exp path)
SCH_A = 128.0 / np.log(2.0)  # Schraudolph bf16 scale
SCH_C = 8.5  # Schraudolph magic (calibrated)
MASK_BIAS = -30000.0  # scalar-path masked bias


def build_nc():
    nc = bacc.Bacc(None, target_bir_lowering=False)

    sqt = nc.declare_dram_parameter("sqt", [NSUB, 8, 128, SW], BF16, isOutput=False)
    skvt = nc.declare_dram_parameter("skvt", [NSUB, 8, 128, SW], BF16, isOutput=False)
    wq = nc.declare_dram_parameter("wq", [8, 128, HPC * DH], BF16, isOutput=False)
    wkv = nc.declare_dram_parameter("wkv", [8, 128, HPC * 2 * DH], BF16, isOutput=False)
    wo = nc.declare_dram_parameter("wo", [2, 128, DOUT], BF16, isOutput=False)
    bq = nc.declare_dram_parameter("bq", [2, 128, 1], F32, isOutput=False)
    bkv = nc.declare_dram_parameter("bkv", [HPC, 128, 1], F32, isOutput=False)
    cost_d = nc.declare_dram_parameter("cost", [128, N], BF16, isOutput=False)
    sint_d = nc.declare_dram_parameter("sint", [128, N], BF16, isOutput=False)
    perm_d = nc.declare_dram_parameter("permm", [128, 128], BF16, isOutput=False)
    negi_d = nc.declare_dram_parameter("negi", [128, 128], BF16, isOutput=False)
    tri_d = nc.declare_dram_parameter("tri", [128, 128], BF16, isOutput=False)
    ident_d = nc.declare_dram_parameter("ident", [128, 128], BF16, isOutput=False)
    bmask_d = nc.declare_dram_parameter("bmask", [NB, 128, 1], F32, isOutput=False)
    b2_d = nc.declare_dram_parameter("b2", [NB, 128, 1], F32, isOutput=False)
    out_ext = nc.declare_dram_parameter("out", [N, DOUT], BF16, isOutput=True)

    AF = mybir.ActivationFunctionType
    ALU = mybir.AluOpType

    with tile.TileContext(nc) as tc:
        with (
            tc.tile_pool(name="const", bufs=1) as cpool,
            tc.tile_pool(name="big", bufs=1) as bigpool,
            tc.tile_pool(name="stream", bufs=1) as spool,
            tc.tile_pool(name="ptile", bufs=1) as ppool,
            tc.tile_pool(name="small", bufs=1) as smallpool,
            tc.tile_pool(name="psA", bufs=1, space=bass.MemorySpace.PSUM) as psA,
            tc.tile_pool(name="psB", bufs=1, space=bass.MemorySpace.PSUM) as psB,
        ):
            # ---- constant tiles ----
            wq_sb = [cpool.tile([128, HPC * DH], BF16, tag=f"wq{c}", name=f"wq{c}") for c in range(8)]
            wkv_sb = [cpool.tile([128, HPC * 2 * DH], BF16, tag=f"wkv{c}", name=f"wkv{c}") for c in range(8)]
            wo_sb = [cpool.tile([128, DOUT], BF16, tag=f"wo{pr}", name=f"wo{pr}") for pr in range(2)]
            bq_sb = cpool.tile([128, 2], F32, tag="bq", name="bq")
            bkv_sb = cpool.tile([128, HPC], F32, tag="bkv", name="bkv")
            cost = cpool.tile([128, N], BF16, tag="cost", name="cost")
            sint = cpool.tile([128, N], BF16, tag="sint", name="sint")
            permm = cpool.tile([128, 128], BF16, tag="permm", name="permm")
            negi = cpool.tile([128, 128], BF16, tag="negi", name="negi")
            tri = cpool.tile([128, 128], BF16, tag="tri", name="tri")
            ident = cpool.tile([128, 128], BF16, tag="ident", name="ident")
            bmask = cpool.tile([128, NB], F32, tag="bmask", name="bmask")
            b2 = cpool.tile([128, NB], F32, tag="b2", name="b2")
            ones1 = cpool.tile([33, 64], BF16, tag="ones1", name="ones1")
            dn = cpool.tile([33, 2, 512], F32, tag="dn", name="dn")

            # ---- persistent activations ----
            qT = [bigpool.tile([128, N], BF16, tag=f"qT{i}", name=f"qT{i}") for i in range(2)]
            kvT = [bigpool.tile([128, N], BF16, tag=f"kvT{h}", name=f"kvT{h}") for h in range(HPC)]
            vg = [
                [bigpool.tile([128, 4, VS], BF16, tag=f"vg{h}_{g}", name=f"vg{h}_{g}") for g in range(NG)]
                for h in range(HPC)
            ]
            oTs = [
                [bigpool.tile([128, 512], BF16, tag=f"oTs{pr}_{g}", name=f"oTs{pr}_{g}") for g in range(NG)]
                for pr in range(2)
            ]

            # PSUM plan (8 banks): oT 4 banks; sT ring 2; shared ring 2
            # (proj chains, rope shuffles, v-transposes, norm broadcast, out-proj).
            oT = psA.tile([65, HPC, 512], F32, tag="oT", name="oT")

            def st_ring():
                return psA.tile([128, 512], F32, tag="sT", name="sT", bufs=3)

            def pj_ring():
                return psB.tile([128, 512], F32, tag="pj", name="pj", bufs=1)

            # ---- DMAs: critical-path first ----
            nc.sync.dma_start(bq_sb[:, 0:1], bq[0])
            nc.sync.dma_start(bq_sb[:, 1:2], bq[1])
            for h in range(HPC):
                nc.sync.dma_start(bkv_sb[:, h : h + 1], bkv[h])

            xqs = [[None] * 8 for _ in range(NSUB)]
            xkvs = [[None] * 8 for _ in range(NSUB)]

            def dma_x(sub, kv):
                for c in range(8):
                    t = spool.tile(
                        [128, SW], BF16, tag="xkv" if kv else "xq", name="x", bufs=32
                    )
                    src = skvt if kv else sqt
                    nc.sync.dma_start(t[:], src[sub, c])
                    if kv:
                        xkvs[sub][c] = t
                    else:
                        xqs[sub][c] = t

            for c in range(8):
                nc.sync.dma_start(wq_sb[c][:], wq[c])
                t = spool.tile([128, SW], BF16, tag="xq", name="x", bufs=32)
                nc.sync.dma_start(t[:], sqt[0, c])
                xqs[0][c] = t
            nc.sync.dma_start(permm[:], perm_d[:])
            nc.sync.dma_start(cost[:], cost_d[:])
            nc.sync.dma_start(sint[:], sint_d[:])
            for c in range(8):
                nc.sync.dma_start(wkv_sb[c][:], wkv[c])
                t = spool.tile([128, SW], BF16, tag="xkv", name="x", bufs=32)
                nc.sync.dma_start(t[:], skvt[0, c])
                xkvs[0][c] = t
            nc.sync.dma_start(negi[:], negi_d[:])
            nc.sync.dma_start(tri[:], tri_d[:])
            nc.sync.dma_start(ident[:], ident_d[:])
            for kb in range(NB):
                nc.sync.dma_start(bmask[:, kb : kb + 1], bmask_d[kb])
                nc.sync.dma_start(b2[:, kb : kb + 1], b2_d[kb])
            dma_x(1, False)
            dma_x(1, True)
            for pr in range(2):
                nc.sync.dma_start(wo_sb[pr][:], wo[pr])
            nc.vector.memset(ones1[:], 1.0)
            nc.vector.memset(dn[:], 1.0)
            for sub in range(2, NSUB):
                dma_x(sub, False)
                dma_x(sub, True)

            # ---------------- emission building blocks ----------------
            def rope_unit(dst, rbase, rows, cs):
                """RoPE in place on dst[rbase:rbase+rows, cs]."""
                sh = pj_ring()
                rsl = slice(rbase, rbase + rows)
                nc.tensor.matmul(
                    sh[rsl, 0:SW],
                    permm[rsl, rbase : rbase + rows],
                    dst[rsl, cs],
                    start=True,
                    stop=True,
                )
                tsb = smallpool.tile([128, SW], BF16, tag="ropet", name="ropet", bufs=2)
                tsc = smallpool.tile([128, SW], BF16, tag="ropec", name="ropec", bufs=2)
                nc.vector.tensor_mul(tsb[rsl, :], sh[rsl, 0:SW], sint[rsl, cs])
                nc.gpsimd.tensor_mul(tsc[rsl, :], dst[rsl, cs], cost[rsl, cs])
                nc.vector.tensor_add(dst[rsl, cs], tsc[rsl, :], tsb[rsl, :])

            def proj_chain(sub, kind, i):
                """Emit one projection chain (8 matmuls + act + rope)."""
                nq0 = sub * SW
                cs = slice(nq0, nq0 + SW)
                ps = pj_ring()
                for c in range(8):
                    w_sb = wq_sb[c] if kind == "q" else wkv_sb[c]
                    x_sb = xqs[sub][c] if kind == "q" else xkvs[sub][c]
                    nc.tensor.matmul(
                        ps[:],
                        w_sb[:, i * 128 : (i + 1) * 128],
                        x_sb[:],
                        start=(c == 0),
                        stop=(c == 7),
                    )
                if kind == "q":
                    nc.scalar.activation(
                        qT[i][:, cs], ps[:], AF.Identity, bias=bq_sb[:, i : i + 1]
                    )
                    rope_unit(qT[i], 0, 128, cs)
                else:
                    nc.scalar.activation(
                        kvT[i][:, cs], ps[:], AF.Identity, bias=bkv_sb[:, i : i + 1]
                    )
                    rope_unit(kvT[i], (i % 2) * 64, 64, cs)

            exp_cnt = [0]

            pend_o = []  # staggered o-matmul emitters (one tile behind)

            def flush_o(n=None):
                k = len(pend_o) if n is None else min(n, len(pend_o))
                for _ in range(k):
                    pend_o.pop(0)()

            def att_tile(g, h, kb):
                """Score + exp now; o-accumulate staggered one tile later."""
                kr = (h % 2) * 64
                pr, hr = h // 2, (h % 2) * 64
                diag = kb // 4 == g
                off = (kb % 4) * 128 if diag else 0
                w = 512 - off
                sT = st_ring()
                nc.tensor.matmul(
                    sT[:, off : off + w],
                    kvT[h][kr : kr + 64, kb * 128 : (kb + 1) * 128],
                    qT[pr][hr : hr + 64, g * 512 + off : g * 512 + off + w],
                    start=True,
                    stop=not diag,
                )
                if diag:
                    nc.tensor.matmul(
                        sT[:, off : off + 128], negi[:], tri[:], start=False, stop=True
                    )
                use_scalar = diag or (exp_cnt[0] % 2 == 0)
                if not diag:
                    exp_cnt[0] += 1
                if use_scalar:
                    p = ppool.tile([128, 512], BF16, tag="p", name="p", bufs=4)
                    nc.scalar.activation(
                        p[:, off : off + w],
                        sT[:, off : off + w],
                        AF.Exp,
                        bias=bmask[:, kb : kb + 1],
                        scale=0.125,
                    )
                    pmv = p[:, off : off + w]
                else:
                    pi = ppool.tile([128, 512], I16, tag="pi", name="pi", bufs=4)
                    nc.vector.tensor_scalar(
                        pi[:, off : off + w],
                        sT[:, off : off + w],
                        0.125 * SCH_A,
                        b2[:, kb : kb + 1],
                        ALU.mult,
                        ALU.add,
                    )
                    pmv = pi[:, off : off + w].bitcast(BF16)

                nc.tensor.matmul(
                    oT[:, h, off : off + w],
                    vg[h][kb // 4][:, kb % 4, 0:65],
                    pmv,
                    start=(kb == 0),
                    stop=(kb == 4 * g + 3),
                )
                if kb == 4 * g + 3:  # head closed: stage denominator row
                    r0 = 32 * (h // 2)
                    nc.vector.tensor_copy(dn[r0 : r0 + 1, h % 2, :], oT[64:65, h, :])

            def norm_group(g):
                flush_o()
                rec = smallpool.tile([33, 2, 512], F32, tag="rec", name="rec", bufs=2)
                recb = smallpool.tile([33, 2, 512], BF16, tag="recb", name="recb", bufs=2)
                nc.vector.reciprocal_approx_fast(rec[:], dn[:])
                nc.vector.tensor_copy(recb[:], rec[:])
                for h in range(HPC):
                    pr, hr = h // 2, (h % 2) * 64
                    r0 = 32 * (h // 2)
                    bc = pj_ring()
                    nc.tensor.matmul(
                        bc[0:64, :],
                        ones1[r0 : r0 + 1, :],
                        recb[r0 : r0 + 1, h % 2, :],
                        start=True,
                        stop=True,
                    )
                    bcs = smallpool.tile([64, 512], F32, tag="bcs", name="bcs", bufs=2)
                    nc.scalar.activation(bcs[:], bc[0:64, :], AF.Copy)
                    nc.vector.tensor_mul(
                        oTs[pr][g][hr : hr + 64, :], oT[0:64, h, :], bcs[:]
                    )

            def po_unit(g, qb, nh, use_st=False):
                def emit():
                    off = (qb % 4) * 128
                    po = st_ring() if use_st else pj_ring()
                    for pr in range(2):
                        nc.tensor.matmul(
                            po[:],
                            oTs[pr][g][:, off : off + 128],
                            wo_sb[pr][:, nh * 512 : (nh + 1) * 512],
                            start=(pr == 0),
                            stop=(pr == 1),
                        )
                    ob = smallpool.tile([128, 512], BF16, tag="ob", name="ob", bufs=4)
                    if (qb + nh) % 2 == 0:
                        nc.scalar.activation(ob[:], po[:], AF.Copy)
                    else:
                        nc.vector.tensor_copy(ob[:], po[:])
                    nc.sync.dma_start(
                        out_ext[qb * 128 : (qb + 1) * 128, nh * 512 : (nh + 1) * 512],
                        ob[:],
                    )
                return emit

            def vprime(g, h):
                vb = 64 if h % 2 == 0 else 0
                nc.gpsimd.memset(vg[h][g][:], 1.0)
                pk = pj_ring()[:, 0:128].bitcast(BF16)  # [128, 256] bf16 view
                for j in range(4):
                    kb = 4 * g + j
                    nc.tensor.matmul(
                        pk[:, j * 64 : (j + 1) * 64],
                        kvT[h][vb : vb + 64, kb * 128 : (kb + 1) * 128],
                        ident[vb : vb + 64, vb : vb + 64],
                        is_transpose=True,
                        start=(j == 0),
                        stop=(j == 3),
                    )
                nc.scalar.activation(vg[h][g][:, :, 0:64], pk[:, 0:256], AF.Copy)

            def att_units(g, heads):
                """Emitter list for attention of group g over given heads,
                each head prefixed by its v'-build."""
                units = []
                for h in heads:
                    units.append(lambda g=g, h=h: vprime(g, h))
                    for kb in range(4 * g + 4):
                        units.append(lambda g=g, h=h, kb=kb: att_tile(g, h, kb))
                return units

            def emit_all(units):
                for u in units:
                    u()

            def interleave(chains, fillers):
                """Emit chain emitters with filler units spread between them."""
                fillers = list(fillers)
                nch = len(chains)
                per = max(1, len(fillers) // max(1, nch)) if fillers else 0
                fi = 0
                for ci, ch in enumerate(chains):
                    ch()
                    take = per if ci < nch - 1 else len(fillers) - fi
                    for _ in range(max(0, take)):
                        if fi < len(fillers):
                            fillers[fi]()
                            fi += 1
                while fi < len(fillers):
                    fillers[fi]()
                    fi += 1

            # ---------------- fused pipeline ----------------
            def chains_for(sub):
                order = [("q", 0), ("kv", 0), ("kv", 1), ("q", 1), ("kv", 2), ("kv", 3)]
                return [lambda k=k, i=i: proj_chain(sub, k, i) for k, i in order]

            # sub 0: chains serialized against DMA arrival; attention g0
            # tiles become available per head as its chains complete.
            a0 = att_units(0, range(HPC))
            interleave(chains_for(0), [])
            emit_all(a0)
            norm_group(0)
            # proj sub1 with po(g0) as fillers (via sT slots; no att tiles live)
            interleave(
                chains_for(1),
                [po_unit(0, qb, nh, use_st=True) for qb in range(0, 4) for nh in range(2)],
            )
            a1 = att_units(1, range(HPC))
            half = len(a1) // 2
            emit_all(a1[:half])
            interleave(chains_for(2), a1[half:])
            norm_group(1)
            a2 = att_units(2, range(HPC))
            third = len(a2) // 3
            interleave([po_unit(1, qb, nh) for qb in range(4, 8) for nh in range(2)], a2[:third])
            interleave(chains_for(3), a2[third:])
            norm_group(2)
            a3 = att_units(3, range(HPC))
            interleave([po_unit(2, qb, nh) for qb in range(8, 12) for nh in range(2)], a3)
            norm_group(3)
            for qb in range(12, 16):
                for nh in range(2):
                    po_unit(3, qb, nh, use_st=(nh % 2 == 1))()

    nc.compile()
    return nc


def _head_perm():
    """Permute one head's 64 dims: [evens(0,2..30), odds(1,3..31), pass 32:64]."""
    ev = np.arange(0, ROT, 2)
    od = np.arange(1, ROT, 2)
    return np.concatenate([ev, od, np.arange(ROT, DH)])


def _prep_host(s_q, s_kv, mask_q, mask_kv, Wq, bq_, Wkv, bkv_, Wo, bo_):
    """Build per-core input maps (host-side shard + transform)."""
    perm = _head_perm()

    # RoPE tables [128, N]: per 64-row block: rows 0:16 evens (cos, +sin),
    # rows 16:32 odds (cos, -sin), rows 32:64 pass (1, 0).
    inv_freq = 1.0 / (10000.0 ** (np.arange(0, ROT, 2, dtype=np.float64) / ROT))
    t = np.arange(N, dtype=np.float64)[None, :] * inv_freq[:, None]  # [16, N]
    cosT = np.zeros((128, N), np.float32)
    sinT = np.zeros((128, N), np.float32)
    cosT[:, :] = 1.0
    for rb in (0, 64):
        cosT[rb : rb + 16] = np.cos(t)
        cosT[rb + 16 : rb + 32] = np.cos(t)
        sinT[rb : rb + 16] = -np.sin(t)
        sinT[rb + 16 : rb + 32] = np.sin(t)
    cosT = cosT.astype(NPBF16)
    sinT = sinT.astype(NPBF16)

    # partner permutation matrix: permm[r, p] = 1 iff r = partner(p)
    pm = np.zeros((128, 128), np.float32)
    for p in range(128):
        b = p % 64
        if b < 16:
            partner = p + 16
        elif b < 32:
            partner = p - 16
        else:
            partner = p
        pm[partner, p] = 1.0
    pm = pm.astype(NPBF16)

    negi = (TRI_C * np.eye(128, dtype=np.float32)).astype(NPBF16)
    ident = np.eye(128, dtype=NPBF16)
    pidx = np.arange(128)
    trim = (pidx[:, None] > pidx[None, :]).astype(np.float32).astype(NPBF16)

    in_maps = []
    for core in range(8):
        b = core // 4
        h0 = (core % 4) * HPC

        wq_cols = []
        bq_cols = []
        for h in range(h0, h0 + HPC):
            wq_cols.append(Wq[:, h * DH : (h + 1) * DH][:, perm])
            bq_cols.append(bq_[h * DH : (h + 1) * DH][perm])
        wq_c = np.concatenate(wq_cols, axis=1)  # [1024, 256]
        bq_c = np.concatenate(bq_cols)  # [256]

        wkv_cols = []
        bkv_cols = []
        for h in range(h0, h0 + HPC):
            kcols = Wkv[:, h * 2 * DH : h * 2 * DH + DH][:, perm]
            vcols = Wkv[:, h * 2 * DH + DH : (h + 1) * 2 * DH]
            kb_ = bkv_[h * 2 * DH : h * 2 * DH + DH][perm]
            vb_ = bkv_[h * 2 * DH + DH : (h + 1) * 2 * DH]
            if (h - h0) % 2 == 0:  # even head: [k; v]
                wkv_cols.append(np.concatenate([kcols, vcols], axis=1))
                bkv_cols.append(np.concatenate([kb_, vb_]))
            else:  # odd head: [v; k] so k-rows sit at partition base 64
                wkv_cols.append(np.concatenate([vcols, kcols], axis=1))
                bkv_cols.append(np.concatenate([vb_, kb_]))
        wkv_c = np.concatenate(wkv_cols, axis=1)  # [1024, 512]

        wo_rows = Wo[h0 * DH : (h0 + HPC) * DH, :]  # [256, 1024]

        braw = 1.0e6 * (mask_kv[b].astype(np.float64) - 1.0)  # 0 or -1e6
        bmask = np.clip(braw, MASK_BIAS, 0.0).reshape(NB, 128, 1)
        b2v = (
            np.clip(SCH_A * braw, -40000.0, 0.0) + (127.0 * 128.0 - SCH_C)
        ).reshape(NB, 128, 1)

        in_maps.append(
            {
                "sqt": np.ascontiguousarray(
                    s_q[b].T.reshape(1024, NSUB, SW).transpose(1, 0, 2).reshape(NSUB, 8, 128, SW)
                ).astype(NPBF16),
                "skvt": np.ascontiguousarray(
                    s_kv[b].T.reshape(1024, NSUB, SW).transpose(1, 0, 2).reshape(NSUB, 8, 128, SW)
                ).astype(NPBF16),
                "wq": np.ascontiguousarray(wq_c.reshape(8, 128, HPC * DH)).astype(NPBF16),
                "wkv": np.ascontiguousarray(wkv_c.reshape(8, 128, HPC * 2 * DH)).astype(NPBF16),
                "wo": np.ascontiguousarray(wo_rows.reshape(2, 128, DOUT)).astype(NPBF16),
                "bq": bq_c.reshape(2, 128, 1).astype(np.float32),
                "bkv": np.stack(bkv_cols).reshape(HPC, 128, 1).astype(np.float32),
                "cost": cosT,
                "sint": sinT,
                "permm": pm,
                "negi": negi,
                "tri": trim,
                "ident": ident,
                "bmask": bmask.astype(np.float32),
                "b2": b2v.astype(np.float32),
            }
        )
    return in_maps


_NC_CACHE = {}


def kernel(s_q, s_kv, mask_q, mask_kv, Wq, bq, Wkv, bkv, Wo, bo, _return_results=False):
    from concourse.bass_utils import run_bass_kernel_spmd

    if "nc" not in _NC_CACHE:
        _NC_CACHE["nc"] = build_nc()
    nc = _NC_CACHE["nc"]

    in_maps = _prep_host(
        np.asarray(s_q, np.float32),
        np.asarray(s_kv, np.float32),
        np.asarray(mask_q, np.float32),
        np.asarray(mask_kv, np.float32),
        np.asarray(Wq, np.float32),
        np.asarray(bq, np.float32),
        np.asarray(Wkv, np.float32),
        np.asarray(bkv, np.float32),
        np.asarray(Wo, np.float32),
        np.asarray(bo, np.float32),
    )
    trace = bool(int(os.environ.get("KERNEL_TRACE", "0")))
    res = run_bass_kernel_spmd(nc, in_maps, core_ids=list(range(8)), trace=trace)

    out = np.zeros((B, N, DOUT), np.float32)
    for core in range(8):
        b = core // 4
        out[b] += res.results[core]["out"].astype(np.float32)
    out += np.asarray(bo, np.float32)[None, None, :]
    if _return_results:
        return out, res
    return out
